# Initial kernel scaffold
#
"""EncoderBlock (QANet-style) Trainium2 Bass kernel, 8-core data parallel.

Full inputs in, full outputs out. Shards batch (1024) across 8 cores.
Per-core: 128 samples, processed in 16 blocks of 8.

Layout: channel-major per sample [D=128 partitions, L=128 free].
- sep_conv fused on PE: h = sum_k (pw . diag(dw_k)) @ shift_k(u), 7 shifted
  accumulating matmuls against a zero-padded u tile.
- LayerNorm over (D,L): per-partition sums/sumsq via stt accum_out,
  cross-partition reduce via ones-matmul on PE, rsqrt via reciprocal+sqrt+Newton,
  broadcast back via K=1 ones matmul, applied with one tensor_scalar op.
- MHA without per-head transposes: S^T = kT_h^T-packed matmuls (row tiling),
  one masked+scaled Exp over all 4 heads, unnormalized O = A^T-lhsT matmuls,
  per-head row normalization folded into the PSUM evacuation.
"""
import sys, os, math

for _p in ("/opt/trn_rl_repo", "/root/.axon_site/_ro/trn_rl_repo"):
    if os.path.isdir(_p) and _p not in sys.path:
        sys.path.append(_p)

os.environ.setdefault("NEURON_RT_RESET_CORES", "1")

import numpy as np
import ml_dtypes

import concourse.bass as bass
import concourse.tile as tile
from concourse import bacc, mybir
from concourse.bass_utils import run_bass_kernel_spmd

F32 = mybir.dt.float32
BF16 = mybir.dt.bfloat16
I32 = mybir.dt.int32
BF = ml_dtypes.bfloat16
ALU = mybir.AluOpType
ACTF = mybir.ActivationFunctionType

D = 128
L = 128
NH = 4
DK = 32
KW = 7
LC = 4
EPS = 1e-5
B = 1024
NCORES = 8
BS = B // NCORES      # 128 samples per core
NB = 8                # samples per block
NBLK = BS // NB       # 16 blocks
KBLKS = int(os.environ.get("KBLKS", NBLK))
PADL = L + KW - 1     # 134, padded conv row
INV_N = 1.0 / (D * L)
MASK_NEG = -100000.0

LAST_RESULT = None


def _pos_encoding():
    i = np.arange(D).astype(np.float64)
    freqs = np.where(i % 2 == 0, 10000.0 ** (-i / D), -(10000.0 ** ((1 - i) / D)))
    phases = np.where(i % 2 == 0, 0.0, math.pi / 2)
    pos = np.arange(L).astype(np.float64)[None, :]
    pe = np.sin(pos * freqs[:, None] + phases[:, None])
    return pe.astype(np.float32)


def _build(weights_np, aff_identity, bv_zero, bo_zero):
    nc = bacc.Bacc("TRN2", target_bir_lowering=False, debug=False)

    xs_d = nc.dram_tensor("xs", [BS, D, L], F32, kind="ExternalInput").ap()
    ms_d = nc.dram_tensor("ms", [BS, L], F32, kind="ExternalInput").ap()
    out_d = nc.dram_tensor("out", [BS, D, L], F32, kind="ExternalOutput").ap()

    cwd = nc.dram_tensor("cw", [D, LC * KW * D], BF16, kind="ExternalInput").ap()
    bcd = nc.dram_tensor("bc", [D, 8], F32, kind="ExternalInput").ap()
    ped = nc.dram_tensor("pe", [D, L], F32, kind="ExternalInput").ap()
    wqd = nc.dram_tensor("wq", [D, D], BF16, kind="ExternalInput").ap()
    wkd = nc.dram_tensor("wk", [D, D], BF16, kind="ExternalInput").ap()
    wvd = nc.dram_tensor("wv", [D, D], BF16, kind="ExternalInput").ap()
    wod = nc.dram_tensor("wo", [D, D], BF16, kind="ExternalInput").ap()
    fcd = nc.dram_tensor("fc", [D, D], BF16, kind="ExternalInput").ap()
    idbd = nc.dram_tensor("idb", [D, D], BF16, kind="ExternalInput").ap()
    idfd = nc.dram_tensor("idf", [NB, NB], F32, kind="ExternalInput").ap()
    on1d = nc.dram_tensor("on1", [1, D], F32, kind="ExternalInput").ap()
    onbd = nc.dram_tensor("onb", [D, 1], BF16, kind="ExternalInput").ap()
    affd = None
    if not all(aff_identity):
        affd = nc.dram_tensor("aff", [D, 12 * L], F32, kind="ExternalInput").ap()
    extd = None
    if not (bv_zero and bo_zero):
        extd = nc.dram_tensor("ext", [D, 2 * L], F32, kind="ExternalInput").ap()

    with tile.TileContext(nc) as tc:
        with (
            tc.tile_pool(name="w", bufs=1) as wp,
            tc.tile_pool(name="xio", bufs=2) as xp,
            tc.tile_pool(name="y", bufs=8) as yp,
            tc.tile_pool(name="u", bufs=3) as up,
            tc.tile_pool(name="u45", bufs=3) as up45,
            tc.tile_pool(name="st", bufs=4) as stp,
            tc.tile_pool(name="tiny", bufs=16) as tp,
            tc.tile_pool(name="mha", bufs=3) as mp,
            tc.tile_pool(name="scr", bufs=3) as scp,
            tc.tile_pool(name="pph", bufs=2, space="PSUM") as pph,
            tc.tile_pool(name="ppq", bufs=2, space="PSUM") as ppq,
            tc.tile_pool(name="ppS", bufs=1, space="PSUM") as ppS,
            tc.tile_pool(name="ppt", bufs=1, space="PSUM") as ppt,
            tc.tile_pool(name="pps", bufs=2, space="PSUM") as pps,
        ):
            # ---- constants to SBUF (once) ----
            cw = wp.tile([D, LC * KW * D], BF16)
            nc.sync.dma_start(cw[:], cwd[:])
            bcol = wp.tile([D, 8], F32)
            nc.sync.dma_start(bcol[:], bcd[:])
            pe_sb = wp.tile([D, L], F32)
            nc.sync.dma_start(pe_sb[:], ped[:])
            wq = wp.tile([D, D], BF16)
            nc.sync.dma_start(wq[:], wqd[:])
            wk = wp.tile([D, D], BF16)
            nc.sync.dma_start(wk[:], wkd[:])
            wv = wp.tile([D, D], BF16)
            nc.sync.dma_start(wv[:], wvd[:])
            wo = wp.tile([D, D], BF16)
            nc.sync.dma_start(wo[:], wod[:])
            fcw = wp.tile([D, D], BF16)
            nc.sync.dma_start(fcw[:], fcd[:])
            idb = wp.tile([D, D], BF16)
            nc.sync.dma_start(idb[:], idbd[:])
            idf = wp.tile([NB, NB], F32)
            nc.sync.dma_start(idf[:], idfd[:])
            on1 = wp.tile([1, D], F32)
            nc.sync.dma_start(on1[:], on1d[:])
            onb = wp.tile([D, 1], BF16)
            nc.sync.dma_start(onb[:], onbd[:])
            aff = None
            if affd is not None:
                aff = wp.tile([D, 12 * L], F32)
                nc.sync.dma_start(aff[:], affd[:])
            ext = None
            if extd is not None:
                ext = wp.tile([D, 2 * L], F32)
                nc.sync.dma_start(ext[:], extd[:])

            ones_f32 = bcol[:, 7:8]  # all-ones fp32 column

            def finalize_norm(stS, stQ):
                """Per-block LN stat finalize: returns sc [128,16] f32
                (cols 0..7 rstd per sample, 8..15 mu*rstd)."""
                pstS = pps.tile([1, NB], F32, tag="ps")
                nc.tensor.matmul(pstS[:], ones_f32, stS[:], start=True, stop=True)
                pstQ = pps.tile([1, NB], F32, tag="ps")
                nc.tensor.matmul(pstQ[:], ones_f32, stQ[:], start=True, stop=True)
                ef = tp.tile([1, 2 * NB], F32, tag="ef")
                nc.vector.tensor_scalar(ef[:, 0:NB], pstS[:], INV_N, None, ALU.mult)
                nc.vector.tensor_scalar(ef[:, NB:2 * NB], pstQ[:], INV_N, None, ALU.mult)
                mu = ef[:, 0:NB]
                ex2 = ef[:, NB:2 * NB]
                mu2 = tp.tile([1, NB], F32, tag="mu2")
                nc.vector.tensor_tensor(mu2[:], mu, mu, ALU.mult)
                ve = tp.tile([1, NB], F32, tag="ve")
                nc.vector.scalar_tensor_tensor(ve[:], ex2, EPS, mu2[:], ALU.add, ALU.subtract)
                rec = tp.tile([1, NB], F32, tag="rec")
                nc.vector.reciprocal(rec[:], ve[:])
                r0 = tp.tile([1, NB], F32, tag="r0")
                nc.scalar.activation(r0[:], rec[:], ACTF.Sqrt)
                t1 = tp.tile([1, NB], F32, tag="t1")
                nc.vector.tensor_tensor(t1[:], r0[:], r0[:], ALU.mult)
                t2 = tp.tile([1, NB], F32, tag="t2")
                nc.vector.tensor_tensor(t2[:], t1[:], ve[:], ALU.mult)
                t3 = tp.tile([1, NB], F32, tag="t3")
                nc.vector.tensor_scalar(t3[:], t2[:], -0.5, 1.5, ALU.mult, ALU.add)
                pack = tp.tile([1, 2 * NB], F32, tag="pack")
                nc.vector.tensor_tensor(pack[:, 0:NB], r0[:], t3[:], ALU.mult)
                nc.vector.tensor_tensor(pack[:, NB:2 * NB], mu, pack[:, 0:NB], ALU.mult)
                pbc = pps.tile([D, 2 * NB], F32, tag="ps")
                nc.tensor.matmul(pbc[:], on1[:], pack[:], start=True, stop=True)
                sc = tp.tile([D, 2 * NB], F32, tag="sc")
                nc.vector.tensor_copy(sc[:], pbc[:])
                return sc

            def apply_norm(dst_ap, y_ap, sc, b, norm_idx):
                """dst = (y - mu) * rstd  [elementwise affine applied if needed]"""
                nc.vector.tensor_scalar(
                    dst_ap, y_ap, sc[:, b:b + 1], sc[:, NB + b:NB + b + 1],
                    ALU.mult, ALU.subtract)
                if aff is not None and not aff_identity[norm_idx]:
                    wslice = aff[:, (2 * norm_idx) * L:(2 * norm_idx + 1) * L]
                    bslice = aff[:, (2 * norm_idx + 1) * L:(2 * norm_idx + 2) * L]
                    nc.vector.tensor_tensor(dst_ap, dst_ap, wslice, ALU.mult)
                    nc.vector.tensor_tensor(dst_ap, dst_ap, bslice, ALU.add)

            for blk in range(KBLKS):
                b0 = blk * NB
                # ---- load x block, mask block ----
                xs = xp.tile([D, NB * L], F32, tag="xs")
                nc.sync.dma_start(
                    xs.rearrange("p (n l) -> p n l", l=L),
                    xs_d[b0:b0 + NB, :, :].rearrange("b d l -> d b l"))
                mk = tp.tile([NB, L], F32, tag="mk")
                nc.sync.dma_start(mk[:], ms_d[b0:b0 + NB, :])
                pmt = pps.tile([D, NB], F32, tag="ps")
                nc.tensor.transpose(pmt[:], mk[:], idf[:])
                mneg = tp.tile([D, NB], F32, tag="mneg")
                nc.scalar.activation(mneg[:], pmt[:], ACTF.Copy, bias=0.0, scale=MASK_NEG)

                # ---- stage 1: y0 = x + pe, stats ----
                y0 = yp.tile([D, NB * L], F32, tag="y")
                stS = stp.tile([D, NB], F32, tag="sS")
                stQ = stp.tile([D, NB], F32, tag="sQ")
                sqs = scp.tile([D, L], BF16, tag="sq")
                for b in range(NB):
                    ysl = y0[:, bass.ts(b, L)]
                    nc.vector.scalar_tensor_tensor(
                        ysl, xs[:, bass.ts(b, L)], 0.0, pe_sb[:],
                        ALU.add, ALU.add, accum_out=stS[:, b:b + 1])
                    nc.vector.scalar_tensor_tensor(
                        sqs[:], ysl, 0.0, ysl, ALU.bypass, ALU.mult,
                        accum_out=stQ[:, b:b + 1])
                sc = finalize_norm(stS, stQ)

                ucur = up.tile([D, NB * PADL], BF16, tag="u")
                nc.vector.memset(ucur[:], 0.0)
                u3 = ucur.rearrange("p (n c) -> p n c", c=PADL)
                for b in range(NB):
                    apply_norm(u3[:, b, (KW // 2):(KW // 2) + L],
                               y0[:, bass.ts(b, L)], sc, b, 0)

                ycur = y0
                # ---- conv layers ----
                for li in range(LC):
                    ynew = yp.tile([D, NB * L], F32, tag="y")
                    stS = stp.tile([D, NB], F32, tag="sS")
                    stQ = stp.tile([D, NB], F32, tag="sQ")
                    uc3 = ucur.rearrange("p (n c) -> p n c", c=PADL)
                    for b in range(NB):
                        ph = pph.tile([D, L], F32, tag="ph")
                        for k in range(KW):
                            nc.tensor.matmul(
                                ph[:], cw[:, bass.ts(li * KW + k, D)],
                                uc3[:, b, k:k + L],
                                start=(k == 0), stop=(k == KW - 1))
                        tr = scp.tile([D, L], F32, tag="tr")
                        nc.scalar.activation(tr[:], ph[:], ACTF.Relu,
                                             bias=bcol[:, 3 + li:4 + li])
                        ysl = ynew[:, bass.ts(b, L)]
                        nc.vector.scalar_tensor_tensor(
                            ysl, tr[:], 0.0, ycur[:, bass.ts(b, L)],
                            ALU.add, ALU.add, accum_out=stS[:, b:b + 1])
                        nc.vector.scalar_tensor_tensor(
                            sqs[:], ysl, 0.0, ysl, ALU.bypass, ALU.mult,
                            accum_out=stQ[:, b:b + 1])
                    sc = finalize_norm(stS, stQ)
                    if li < LC - 1:
                        unew = up.tile([D, NB * PADL], BF16, tag="u")
                        nc.vector.memset(unew[:], 0.0)
                        un3 = unew.rearrange("p (n c) -> p n c", c=PADL)
                        for b in range(NB):
                            apply_norm(un3[:, b, (KW // 2):(KW // 2) + L],
                                       ynew[:, bass.ts(b, L)], sc, b, 1 + li)
                        ucur = unew
                    else:
                        u4 = up45.tile([D, NB * L], BF16, tag="u4")
                        for b in range(NB):
                            apply_norm(u4[:, bass.ts(b, L)],
                                       ynew[:, bass.ts(b, L)], sc, b, 1 + li)
                    ycur = ynew

                if os.environ.get("KSTAGE", "full") == "conv":
                    nc.sync.dma_start(
                        out_d[b0:b0 + NB, :, :].rearrange("b d l -> d b l"),
                        ycur.rearrange("p (n l) -> p n l", l=L))
                    continue
                # ---- MHA ----
                ynew = yp.tile([D, NB * L], F32, tag="y")
                stS = stp.tile([D, NB], F32, tag="sS")
                stQ = stp.tile([D, NB], F32, tag="sQ")
                for b in range(NB):
                    put = ppt.tile([D, D], BF16, tag="pt")
                    nc.tensor.transpose(put[:], u4[:, bass.ts(b, L)], idb[:])
                    uT = mp.tile([D, D], BF16, tag="uT")
                    nc.scalar.activation(uT[:], put[:], ACTF.Copy)
                    pq = ppq.tile([D, D], F32, tag="pq")
                    nc.tensor.matmul(pq[:], wq[:], uT[:], start=True, stop=True)
                    qh = [mp.tile([DK, D], BF16, tag=f"qh{h}", name=f"qh{h}_{blk}_{b}") for h in range(NH)]
                    for h in range(NH):
                        nc.scalar.activation(qh[h][:], pq[DK * h:DK * (h + 1), :],
                                             ACTF.Identity, bias=bcol[DK * h:DK * (h + 1), 0:1])
                    pk = ppq.tile([D, D], F32, tag="pq")
                    nc.tensor.matmul(pk[:], wk[:], uT[:], start=True, stop=True)
                    kh = [mp.tile([DK, D], BF16, tag=f"kh{h}", name=f"kh{h}_{blk}_{b}") for h in range(NH)]
                    for h in range(NH):
                        nc.scalar.activation(kh[h][:], pk[DK * h:DK * (h + 1), :],
                                             ACTF.Identity, bias=bcol[DK * h:DK * (h + 1), 1:2])
                    pv = ppq.tile([D, D], F32, tag="pq")
                    nc.tensor.matmul(pv[:], uT[:], wv[:], start=True, stop=True)
                    vsb = mp.tile([D, D], BF16, tag="v")
                    if bv_zero:
                        nc.scalar.activation(vsb[:], pv[:], ACTF.Copy)
                    else:
                        nc.vector.scalar_tensor_tensor(
                            vsb[:], pv[:], 0.0, ext[:, 0:L], ALU.add, ALU.add)
                    pS = ppS.tile([D, NH * L], F32, tag="pS")
                    for h in range(NH):
                        nc.tensor.matmul(
                            pS[:, bass.ts(h, L)], kh[h][:], qh[h][:],
                            start=True, stop=True)
                    asb = mp.tile([D, NH * L], BF16, tag="asb")
                    nc.scalar.activation(asb[:], pS[:], ACTF.Exp,
                                         bias=mneg[:, b:b + 1],
                                         scale=1.0 / math.sqrt(DK))
                    pr = pps.tile([D, NH], F32, tag="ps")
                    pO = ppt.tile([D, D], F32, tag="pt")
                    for h in range(NH):
                        nc.tensor.matmul(pr[:, h:h + 1], asb[:, bass.ts(h, L)],
                                         onb[:], start=True, stop=True)
                        nc.tensor.matmul(pO[:, bass.ts(h, DK)],
                                         asb[:, bass.ts(h, L)],
                                         vsb[:, bass.ts(h, DK)],
                                         start=True, stop=True)
                    r4 = tp.tile([D, NH], F32, tag="r4")
                    nc.vector.reciprocal(r4[:], pr[:])
                    Osb = mp.tile([D, D], BF16, tag="O")
                    for h in range(NH):
                        nc.vector.tensor_scalar(
                            Osb[:, bass.ts(h, DK)], pO[:, bass.ts(h, DK)],
                            r4[:, h:h + 1], None, ALU.mult)
                    pot = ppt.tile([D, D], BF16, tag="pt")
                    nc.tensor.transpose(pot[:], Osb[:], idb[:])
                    OT = mp.tile([D, D], BF16, tag="OT")
                    nc.scalar.activation(OT[:], pot[:], ACTF.Copy)
                    pat = ppq.tile([D, D], F32, tag="pq")
                    nc.tensor.matmul(pat[:], OT[:], wo[:], start=True, stop=True)
                    ysl = ynew[:, bass.ts(b, L)]
                    if bo_zero:
                        nc.vector.scalar_tensor_tensor(
                            ysl, pat[:], 0.0, ycur[:, bass.ts(b, L)],
                            ALU.add, ALU.add, accum_out=stS[:, b:b + 1])
                    else:
                        tb = scp.tile([D, L], F32, tag="tr")
                        nc.vector.tensor_tensor(tb[:], pat[:], ext[:, L:2 * L], ALU.add)
                        nc.vector.scalar_tensor_tensor(
                            ysl, tb[:], 0.0, ycur[:, bass.ts(b, L)],
                            ALU.add, ALU.add, accum_out=stS[:, b:b + 1])
                    nc.vector.scalar_tensor_tensor(
                        sqs[:], ysl, 0.0, ysl, ALU.bypass, ALU.mult,
                        accum_out=stQ[:, b:b + 1])
                sc = finalize_norm(stS, stQ)
                u5 = up45.tile([D, NB * L], BF16, tag="u5")
                for b in range(NB):
                    apply_norm(u5[:, bass.ts(b, L)], ynew[:, bass.ts(b, L)], sc, b, 5)
                ycur = ynew

                # ---- fc + relu + residual -> out ----
                ostage = xp.tile([D, NB * L], F32, tag="os")
                for b in range(NB):
                    pf = pph.tile([D, L], F32, tag="ph")
                    nc.tensor.matmul(pf[:], fcw[:], u5[:, bass.ts(b, L)],
                                     start=True, stop=True)
                    tr = scp.tile([D, L], F32, tag="tr")
                    nc.scalar.activation(tr[:], pf[:], ACTF.Relu, bias=bcol[:, 2:3])
                    nc.vector.tensor_tensor(ostage[:, bass.ts(b, L)], tr[:],
                                            ycur[:, bass.ts(b, L)], ALU.add)
                nc.sync.dma_start(
                    out_d[b0:b0 + NB, :, :].rearrange("b d l -> d b l"),
                    ostage.rearrange("p (n l) -> p n l", l=L))

    nc.compile()
    return nc


def kernel(x, mask, dw_w, dw_b, pw_w, pw_b, normb_w, normb_b, norms_w, norms_b,
           norme_w, norme_b, Wq, bq, Wk, bk, Wv, bv, Wo, bo, fc_w, fc_b):
    global LAST_RESULT
    x = np.asarray(x, np.float32)
    mask = np.asarray(mask, np.int32)
    f = lambda a: np.asarray(a, np.float32)
    dw_w, dw_b, pw_w, pw_b = f(dw_w), f(dw_b), f(pw_w), f(pw_b)
    Wq, bq, Wk, bk, Wv, bv, Wo, bo = f(Wq), f(bq), f(Wk), f(bk), f(Wv), f(bv), f(Wo), f(bo)
    fc_w, fc_b = f(fc_w), f(fc_b)
    normb_w, normb_b = f(normb_w), f(normb_b)
    norms_w, norms_b = f(norms_w), f(norms_b)
    norme_w, norme_b = f(norme_w), f(norme_b)

    # ---- host-side constant prep ----
    pe = _pos_encoding()
    cw = np.zeros((D, LC * KW * D), BF)
    for i in range(LC):
        for k in range(KW):
            mk = pw_w[i] * dw_w[i][:, k][None, :]        # [o, c]
            cw[:, (i * KW + k) * D:(i * KW + k + 1) * D] = mk.T.astype(BF)
    bcols = np.zeros((D, 8), np.float32)
    bcols[:, 0] = bq
    bcols[:, 1] = bk
    bcols[:, 2] = fc_b
    for i in range(LC):
        bcols[:, 3 + i] = pw_w[i] @ dw_b[i] + pw_b[i]
    bcols[:, 7] = 1.0

    aw = [normb_w] + [norms_w[i] for i in range(LC)] + [norme_w]
    ab = [normb_b] + [norms_b[i] for i in range(LC)] + [norme_b]
    aff_identity = [bool(np.all(w == 1.0) and np.all(b == 0.0))
                    for w, b in zip(aw, ab)]
    bv_zero = bool(np.all(bv == 0.0))
    bo_zero = bool(np.all(bo == 0.0))

    nc = _build(None, aff_identity, bv_zero, bo_zero)

    wmap = {
        "cw": cw,
        "bc": bcols,
        "pe": pe,
        "wq": Wq.T.astype(BF),
        "wk": Wk.T.astype(BF),
        "wv": Wv.T.astype(BF),
        "wo": Wo.T.astype(BF),
        "fc": fc_w.T.astype(BF),
        "idb": np.eye(D, dtype=BF),
        "idf": np.eye(NB, dtype=np.float32),
        "on1": np.ones((1, D), np.float32),
        "onb": np.ones((D, 1), BF),
    }
    if not all(aff_identity):
        affarr = np.zeros((D, 12 * L), np.float32)
        for i in range(6):
            affarr[:, 2 * i * L:(2 * i + 1) * L] = aw[i]
            affarr[:, (2 * i + 1) * L:(2 * i + 2) * L] = ab[i]
        wmap["aff"] = affarr
    if not (bv_zero and bo_zero):
        extarr = np.zeros((D, 2 * L), np.float32)
        extarr[:, 0:L] = np.broadcast_to(bv[None, :], (D, L))
        extarr[:, L:2 * L] = np.broadcast_to(bo[None, :], (D, L))
        wmap["ext"] = extarr

    core_ids = list(range(NCORES))
    in_maps = []
    for c in core_ids:
        m = dict(wmap)
        m["xs"] = np.ascontiguousarray(x[c * BS:(c + 1) * BS])
        m["ms"] = np.ascontiguousarray(mask[c * BS:(c + 1) * BS]).astype(np.float32)
        in_maps.append(m)

    if os.environ.get("KSIM", "0") == "1":
        from concourse.bass_interp import CoreSim
        sim = CoreSim(nc, require_finite=False, require_nnan=False)
        for k, v in in_maps[0].items():
            sim.tensor(k)[:] = v
        sim.simulate(check_with_hw=False)
        out0 = np.array(sim.tensor("out"))
        out = np.concatenate([out0] + [np.zeros_like(out0)] * (NCORES - 1), axis=0)
        return out.astype(np.float32)
    trace = os.environ.get("KERNEL_TRACE", "0") == "1"
    try:
        try:
            res = run_bass_kernel_spmd(nc, in_maps, core_ids, trace=trace)
        except Exception:
            if not trace:
                raise
            sys.stderr.write("traced run failed; retrying without trace\n")
            res = run_bass_kernel_spmd(nc, in_maps, core_ids, trace=False)
        LAST_RESULT = res
        out = np.concatenate([res.results[c]["out"] for c in range(NCORES)], axis=0)
        return out.astype(np.float32)
    except Exception as e:
        sys.stderr.write(f"HW path failed ({e}); falling back to CoreSim\n")
        from concourse.bass_interp import CoreSim
        outs = []
        for c in range(NCORES):
            sim = CoreSim(nc, require_finite=False, require_nnan=False)
            for k, v in in_maps[c].items():
                sim.tensor(k)[:] = v
            sim.simulate(check_with_hw=False)
            outs.append(np.array(sim.tensor("out")))
        return np.concatenate(outs, axis=0).astype(np.float32)



# revision 15
# speedup vs baseline: 1.3022x; 1.3022x over previous
"""EncoderBlock (QANet-style) Trainium2 Bass kernel, 8-core data parallel.

Full inputs in, full outputs out. Shards batch (1024) across 8 cores.
Per-core: 128 samples, processed in 16 blocks of 8 (NB=8).

Block-batched layout [D=128 partitions, NB*L free]:
- sep_conv: k-outer accumulation, 7 matmuls of N=1024 (8 samples wide)
  against a zero-gutter padded u tile; weights folded (pw . diag(dw_k)).
- LayerNorm over (D,L): free-dim per-sample sums via one segmented
  tensor_reduce (stS) + per-sample square ops w/ accum_out (stQ, split
  ACT/GpSimd); cross-partition reduce + broadcast on PE.
- MHA: batched uT/OT transposes into one bf16 PSUM bank, batched N=1024
  QKV/Wo projections, row-tiled 4-head score matmuls (K=32 via
  tile_position), AV with a ones-column appended to V so the softmax
  denominator rides the same matmul.
"""
import sys, os, math

for _p in ("/opt/trn_rl_repo", "/root/.axon_site/_ro/trn_rl_repo"):
    if os.path.isdir(_p) and _p not in sys.path:
        sys.path.append(_p)

os.environ.setdefault("NEURON_RT_RESET_CORES", "1")

import numpy as np
import ml_dtypes

import concourse.bass as bass
import concourse.tile as tile
from concourse import bacc, mybir
from concourse.bass_utils import run_bass_kernel_spmd

F32 = mybir.dt.float32
BF16 = mybir.dt.bfloat16
I32 = mybir.dt.int32
BF = ml_dtypes.bfloat16
ALU = mybir.AluOpType
ACTF = mybir.ActivationFunctionType

D = 128
L = 128
NH = 4
DK = 32
KW = 7
LC = 4
EPS = 1e-5
B = 1024
NCORES = 8
BS = B // NCORES      # 128 samples per core
NB = 8                # samples per block
NBLK = BS // NB       # 16 blocks
KBLKS = int(os.environ.get("KBLKS", NBLK))
KLVL = int(os.environ.get("KLVL", "9"))  # bisect: 1=conv 2=qkv 3=scores 9=full
KHEADS = int(os.environ.get("KHEADS", str(NH)))
PADL = L + KW - 1     # 134 cols per sample in padded u tiles
INV_N = 1.0 / (D * L)
MASK_NEG = -100000.0
ISCALE = 1.0 / math.sqrt(DK)

LAST_RESULT = None


def _pos_encoding():
    i = np.arange(D).astype(np.float64)
    freqs = np.where(i % 2 == 0, 10000.0 ** (-i / D), -(10000.0 ** ((1 - i) / D)))
    phases = np.where(i % 2 == 0, 0.0, math.pi / 2)
    pos = np.arange(L).astype(np.float64)[None, :]
    pe = np.sin(pos * freqs[:, None] + phases[:, None])
    return pe.astype(np.float32)


def _build(aff_identity, bv_zero, bo_zero):
    nc = bacc.Bacc("TRN2", target_bir_lowering=False, debug=False)

    xs_d = nc.dram_tensor("xs", [BS, D, L], F32, kind="ExternalInput").ap()
    ms_d = nc.dram_tensor("ms", [BS, L], F32, kind="ExternalInput").ap()
    out_d = nc.dram_tensor("out", [BS, D, L], F32, kind="ExternalOutput").ap()

    cwd = nc.dram_tensor("cw", [D, LC * KW * D], BF16, kind="ExternalInput").ap()
    bcd = nc.dram_tensor("bc", [D, 8], F32, kind="ExternalInput").ap()
    ped = nc.dram_tensor("pe", [D, L], F32, kind="ExternalInput").ap()
    wqd = nc.dram_tensor("wq", [D, D], BF16, kind="ExternalInput").ap()
    wkd = nc.dram_tensor("wk", [D, D], BF16, kind="ExternalInput").ap()
    wvd = nc.dram_tensor("wv", [D, D], BF16, kind="ExternalInput").ap()
    wod = nc.dram_tensor("wo", [D, D], BF16, kind="ExternalInput").ap()
    fcd = nc.dram_tensor("fc", [D, D], BF16, kind="ExternalInput").ap()
    idbd = nc.dram_tensor("idb", [D, D], BF16, kind="ExternalInput").ap()
    idfd = nc.dram_tensor("idf", [NB, NB], F32, kind="ExternalInput").ap()
    on1d = nc.dram_tensor("on1", [1, D], F32, kind="ExternalInput").ap()
    affd = None
    if not all(aff_identity):
        affd = nc.dram_tensor("aff", [D, 12 * L], F32, kind="ExternalInput").ap()
    extd = None
    if not (bv_zero and bo_zero):
        extd = nc.dram_tensor("ext", [D, 2 * L], F32, kind="ExternalInput").ap()

    with tile.TileContext(nc) as tc:
        with (
            tc.tile_pool(name="w", bufs=1) as wp,
            tc.tile_pool(name="xio", bufs=3) as xp,
            tc.tile_pool(name="y", bufs=3) as yp,
            tc.tile_pool(name="u", bufs=3) as up,
            tc.tile_pool(name="tr", bufs=2) as trp,
            tc.tile_pool(name="st", bufs=4) as stp,
            tc.tile_pool(name="sq", bufs=2) as sqp,
            tc.tile_pool(name="tiny", bufs=8) as tp,
            tc.tile_pool(name="mha", bufs=2) as mp,
            tc.tile_pool(name="pbig", bufs=2, space="PSUM") as ppb,
            tc.tile_pool(name="ptr", bufs=2, space="PSUM") as ppt,
            tc.tile_pool(name="psm", bufs=2, space="PSUM") as pps,
        ):
            # ---- constants to SBUF (once) ----
            cw = wp.tile([D, LC * KW * D], BF16)
            nc.sync.dma_start(cw[:], cwd[:])
            bcol = wp.tile([D, 8], F32)
            nc.sync.dma_start(bcol[:], bcd[:])
            wq = wp.tile([D, D], BF16)
            nc.sync.dma_start(wq[:], wqd[:])
            wk = wp.tile([D, D], BF16)
            nc.sync.dma_start(wk[:], wkd[:])
            wv = wp.tile([D, D], BF16)
            nc.sync.dma_start(wv[:], wvd[:])
            wo = wp.tile([D, D], BF16)
            nc.sync.dma_start(wo[:], wod[:])
            fcw = wp.tile([D, D], BF16)
            nc.sync.dma_start(fcw[:], fcd[:])
            idb = wp.tile([D, D], BF16)
            nc.sync.dma_start(idb[:], idbd[:])
            idf = wp.tile([NB, NB], F32)
            nc.sync.dma_start(idf[:], idfd[:])
            on1 = wp.tile([1, D], F32)
            nc.sync.dma_start(on1[:], on1d[:])
            pe_w = wp.tile([D, NB * L], F32)
            for b in range(NB):
                nc.sync.dma_start(pe_w[:, bass.ts(b, L)], ped[:])
            aff = None
            if affd is not None:
                aff = wp.tile([D, 12 * L], F32)
                nc.sync.dma_start(aff[:], affd[:])
            ext = None
            if extd is not None:
                ext = wp.tile([D, 2 * L], F32)
                nc.sync.dma_start(ext[:], extd[:])

            ones_f32 = bcol[:, 7:8]  # all-ones fp32 column
            HB = NB // 2  # samples per psum bank (f32 bank = 512 cols)

            def mm_banked(out3, lhsT, rhs3, start, stop):
                """matmul with [D, NB, L] out split at the psum bank boundary."""
                nc.tensor.matmul(out3[:, 0:HB, :], lhsT, rhs3[:, 0:HB, :],
                                 start=start, stop=stop)
                nc.tensor.matmul(out3[:, HB:NB, :], lhsT, rhs3[:, HB:NB, :],
                                 start=start, stop=stop)

            def finalize_norm(st):
                """st [D, 2*NB] f32 (cols 0:8 sums, 8:16 sumsq) ->
                sc [D, 2*NB] f32 (cols 0:8 rstd r, 8:16 -mu*r)."""
                pst = pps.tile([1, 2 * NB], F32, tag="ps", name="pst")
                nc.tensor.matmul(pst[:], ones_f32, st[:], start=True, stop=True)
                ef = tp.tile([1, 2 * NB], F32, tag="ef")
                nc.vector.tensor_scalar(ef[:], pst[:], INV_N, None, ALU.mult)
                mu = ef[:, 0:NB]
                ex2 = ef[:, NB:2 * NB]
                mu2 = tp.tile([1, NB], F32, tag="mu2")
                nc.vector.tensor_tensor(mu2[:], mu, mu, ALU.mult)
                ve = tp.tile([1, NB], F32, tag="ve")
                nc.vector.scalar_tensor_tensor(ve[:], ex2, EPS, mu2[:], ALU.add,
                                               ALU.subtract)
                rec = tp.tile([1, NB], F32, tag="rec")
                nc.vector.reciprocal(rec[:], ve[:])
                r0 = tp.tile([1, NB], F32, tag="r0")
                nc.scalar.activation(r0[:], rec[:], ACTF.Sqrt)
                # one Newton step: r = r0 * (1.5 - 0.5 * r0^2 * ve)
                t1 = tp.tile([1, NB], F32, tag="t1")
                nc.vector.tensor_tensor(t1[:], r0[:], r0[:], ALU.mult)
                t2 = tp.tile([1, NB], F32, tag="t2")
                nc.vector.tensor_tensor(t2[:], t1[:], ve[:], ALU.mult)
                t3 = tp.tile([1, NB], F32, tag="t3")
                nc.vector.tensor_scalar(t3[:], t2[:], -0.5, 1.5, ALU.mult,
                                        ALU.add)
                pack = tp.tile([1, 2 * NB], F32, tag="pack")
                nc.vector.tensor_tensor(pack[:, 0:NB], r0[:], t3[:], ALU.mult)
                nc.vector.scalar_tensor_tensor(pack[:, NB:2 * NB], mu, -1.0,
                                               pack[:, 0:NB], ALU.mult, ALU.mult)
                pbc = pps.tile([D, 2 * NB], F32, tag="ps", name="pbc")
                nc.tensor.matmul(pbc[:], on1[:], pack[:], start=True, stop=True)
                sc = tp.tile([D, 2 * NB], F32, tag="sc")
                nc.scalar.activation(sc[:], pbc[:], ACTF.Copy)
                return sc

            def stats_sq(st, y3, blk, si):
                """Sum-of-squares per sample into st[:, 8:16]: one batched
                ACT Square then one segmented DVE reduce."""
                sqb = sqp.tile([D, NB * L], BF16, tag="sqb",
                               name=f"sqb_{blk}_{si}")
                sqb3 = sqb.rearrange("p (n l) -> p n l", l=L)
                nc.scalar.activation(sqb3, y3, ACTF.Square)
                nc.vector.tensor_reduce(st[:, NB:2 * NB], sqb3,
                                        mybir.AxisListType.X, ALU.add)

            def apply_norm(udst3, interior, y3, sc, norm_idx, blk):
                """u[b] = y[b]*r_b + m2_b  (bf16), split DVE/ACT/POOL.
                udst3: [D, NB, cols] view; interior: slice within cols."""
                for b in range(NB):
                    dst = udst3[:, b, interior]
                    ysl = y3[:, b, :]
                    r = sc[:, b:b + 1]
                    m2 = sc[:, NB + b:NB + b + 1]
                    if b < 3:
                        nc.vector.tensor_scalar(dst, ysl, r, m2, ALU.mult,
                                                ALU.add)
                    elif b < 6:
                        nc.scalar.activation(dst, ysl, ACTF.Identity,
                                             bias=m2, scale=r)
                    else:
                        nc.gpsimd.tensor_scalar(dst, ysl, r, m2, ALU.mult,
                                                ALU.add)
                if aff is not None and not aff_identity[norm_idx]:
                    wsl = aff[:, (2 * norm_idx) * L:(2 * norm_idx + 1) * L]
                    bsl = aff[:, (2 * norm_idx + 1) * L:(2 * norm_idx + 2) * L]
                    for b in range(NB):
                        dst = udst3[:, b, interior]
                        nc.vector.tensor_tensor(dst, dst, wsl, ALU.mult)
                        nc.vector.tensor_tensor(dst, dst, bsl, ALU.add)

            def new_padded_u(name):
                u = up.tile([D, NB * PADL], BF16, tag="u", name=name)
                u3 = u.rearrange("p (n c) -> p n c", c=PADL)
                nc.vector.memset(u3[:, :, 0:KW // 2], 0.0)
                nc.vector.memset(u3[:, :, KW // 2 + L:PADL], 0.0)
                return u, u3

            for blk in range(KBLKS):
                b0 = blk * NB
                # ---- load x block, mask block ----
                xs = xp.tile([D, NB * L], F32, tag="xs")
                nc.sync.dma_start(
                    xs.rearrange("p (n l) -> p n l", l=L),
                    xs_d[b0:b0 + NB, :, :].rearrange("b d l -> d b l"))
                mk = tp.tile([NB, L], F32, tag="mk")
                nc.sync.dma_start(mk[:], ms_d[b0:b0 + NB, :])
                pmt = pps.tile([D, NB], F32, tag="ps", name="pmt")
                nc.tensor.transpose(pmt[:], mk[:], idf[:])
                mneg = tp.tile([D, NB], F32, tag="mneg")
                nc.scalar.activation(mneg[:], pmt[:], ACTF.Copy, bias=0.0,
                                     scale=MASK_NEG)

                # ---- stage 0: y0 = x + pe ----
                y = yp.tile([D, NB * L], F32, tag="y", name=f"y0_{blk}")
                nc.gpsimd.tensor_tensor(y[:], xs[:], pe_w[:], ALU.add)
                y3 = y.rearrange("p (n l) -> p n l", l=L)
                st = stp.tile([D, 2 * NB], F32, tag="st", name=f"st0_{blk}")
                nc.vector.tensor_reduce(st[:, 0:NB], y3,
                                        mybir.AxisListType.X, ALU.add)
                stats_sq(st, y3, blk, 0)
                sc = finalize_norm(st)
                ucur, uc3 = new_padded_u(f"u0_{blk}")
                apply_norm(uc3, slice(KW // 2, KW // 2 + L), y3, sc, 0, blk)
                ycur = y

                # ---- conv layers ----
                for li in range(LC):
                    ph = ppb.tile([D, NB * L], F32, tag="ph",
                                  name=f"ph_{blk}_{li}")
                    ph3 = ph.rearrange("p (n l) -> p n l", l=L)
                    for k in range(KW):
                        mm_banked(ph3, cw[:, bass.ts(li * KW + k, D)],
                                  uc3[:, :, k:k + L],
                                  start=(k == 0), stop=(k == KW - 1))
                    tr = trp.tile([D, NB * L], BF16, tag="tr",
                                  name=f"tr_{blk}_{li}")
                    nc.scalar.activation(tr[:], ph[:], ACTF.Relu,
                                         bias=bcol[:, 3 + li:4 + li])
                    ynew = yp.tile([D, NB * L], F32, tag="y",
                                   name=f"y{li + 1}_{blk}")
                    nc.gpsimd.tensor_tensor(ynew[:], tr[:], ycur[:], ALU.add)
                    y3 = ynew.rearrange("p (n l) -> p n l", l=L)
                    st = stp.tile([D, 2 * NB], F32, tag="st",
                                  name=f"st{li + 1}_{blk}")
                    nc.vector.tensor_reduce(st[:, 0:NB], y3,
                                            mybir.AxisListType.X, ALU.add)
                    stats_sq(st, y3, blk, 1 + li)
                    sc = finalize_norm(st)
                    if li < LC - 1:
                        ucur, uc3 = new_padded_u(f"u{li + 1}_{blk}")
                        apply_norm(uc3, slice(KW // 2, KW // 2 + L), y3, sc,
                                   1 + li, blk)
                    else:
                        u4 = mp.tile([D, NB * L], BF16, tag="u4",
                                     name=f"u4_{blk}")
                        u43 = u4.rearrange("p (n l) -> p n l", l=L)
                        apply_norm(u43, slice(0, L), y3, sc, 1 + li, blk)
                    ycur = ynew

                if KLVL <= 1:
                    nc.sync.dma_start(
                        out_d[b0:b0 + NB, :, :].rearrange("b d l -> d b l"),
                        ycur.rearrange("p (n l) -> p n l", l=L))
                    continue

                # ---- MHA ----
                # uT: all 8 samples transposed into one bf16 psum bank
                ptu = ppt.tile([D, NB * L], BF16, tag="pt", name=f"ptu_{blk}")
                for b in range(NB):
                    nc.tensor.transpose(ptu[:, bass.ts(b, L)],
                                        u4[:, bass.ts(b, L)], idb[:])
                uT = mp.tile([D, NB * L], BF16, tag="uT", name=f"uT_{blk}")
                nc.scalar.activation(uT[:], ptu[:], ACTF.Copy)

                # Q, K projections batched (N=1024); V per-sample (lhsT=uT_b)
                pq = ppb.tile([D, NB * L], F32, tag="ph", name=f"pq_{blk}")
                mm_banked(pq.rearrange("p (n l) -> p n l", l=L), wq[:],
                          uT.rearrange("p (n l) -> p n l", l=L),
                          start=True, stop=True)
                qsb = mp.tile([D, NB * L], BF16, tag="qsb", name=f"qsb_{blk}")
                nc.scalar.activation(qsb[:], pq[:], ACTF.Identity,
                                     bias=bcol[:, 0:1])
                pk = ppb.tile([D, NB * L], F32, tag="ph", name=f"pk_{blk}")
                mm_banked(pk.rearrange("p (n l) -> p n l", l=L), wk[:],
                          uT.rearrange("p (n l) -> p n l", l=L),
                          start=True, stop=True)
                ksb = mp.tile([D, NB * L], BF16, tag="ksb", name=f"ksb_{blk}")
                nc.scalar.activation(ksb[:], pk[:], ACTF.Identity,
                                     bias=bcol[:, 1:2])
                # regroup Q/K head slices to partitions 0-31 via SBUF DMA so
                # score matmuls run on the default full-array tile (row-tiled
                # matmuls sharing one PSUM bank collide fatally on HW).
                qh = mp.tile([DK, NH * NB * L], BF16, tag="qh",
                             name=f"qh_{blk}")
                kh = mp.tile([DK, NH * NB * L], BF16, tag="kh",
                             name=f"kh_{blk}")
                qh3 = qh.rearrange("p (h n l) -> p h n l", h=NH, l=L)
                kh3 = kh.rearrange("p (h n l) -> p h n l", h=NH, l=L)
                for h in range(NH):
                    nc.sync.dma_start(qh3[:, h, :, :],
                                      qsb[DK * h:DK * (h + 1), :]
                                      .rearrange("p (n l) -> p n l", l=L))
                    nc.sync.dma_start(kh3[:, h, :, :],
                                      ksb[DK * h:DK * (h + 1), :]
                                      .rearrange("p (n l) -> p n l", l=L))
                pv = ppb.tile([D, NB * L], F32, tag="ph", name=f"pv_{blk}")
                for b in range(NB):
                    nc.tensor.matmul(pv[:, bass.ts(b, L)],
                                     uT[:, bass.ts(b, L)], wv[:],
                                     start=True, stop=True)
                # vext [key, (b, h, dk|1)]: V columns + ones column per head
                vext = mp.tile([D, NB * NH * (DK + 1)], BF16, tag="vext",
                               name=f"vext_{blk}")
                v4 = vext.rearrange("p (n h c) -> p n h c", h=NH, c=DK + 1)
                nc.vector.memset(v4[:, :, :, DK:DK + 1], 1.0)
                pv4 = pv.rearrange("p (n h c) -> p n h c", h=NH, c=DK)
                nc.scalar.activation(v4[:, :, :, 0:DK], pv4, ACTF.Copy)
                if ext is not None and not bv_zero:
                    for b in range(NB):
                        nc.vector.tensor_tensor(
                            v4[:, b, :, 0:DK],
                            v4[:, b, :, 0:DK],
                            ext[:, 0:L].rearrange("p (h c) -> p h c", c=DK),
                            ALU.add)

                if KLVL <= 2:
                    nc.sync.dma_start(
                        out_d[b0:b0 + NB, :, :].rearrange("b d l -> d b l"),
                        ycur.rearrange("p (n l) -> p n l", l=L))
                    continue

                # scores + softmax + AV per sample
                ptO = ppt.tile([D, NB * L], BF16, tag="pt", name=f"ptO_{blk}")
                for b in range(NB):
                    pS = pps.tile([D, NH * L], F32, tag="ps",
                                  name=f"pS_{blk}_{b}")
                    for h in range(KHEADS):
                        nc.tensor.matmul(
                            pS[:, bass.ts(h, L)],
                            kh3[:, h, b, :], qh3[:, h, b, :],
                            start=True, stop=True)
                    asb = mp.tile([D, NH * L], BF16, tag="asb",
                                  name=f"asb_{blk}_{b}")
                    nc.scalar.activation(asb[:], pS[:], ACTF.Exp,
                                         bias=mneg[:, b:b + 1], scale=ISCALE)
                    if KLVL <= 3:
                        continue
                    pO = pps.tile([D, NH * (DK + 1)], F32, tag="ps",
                                  name=f"pO_{blk}_{b}")
                    for h in range(NH):
                        nc.tensor.matmul(pO[:, bass.ts(h, DK + 1)],
                                         asb[:, bass.ts(h, L)],
                                         v4[:, b, h, :],
                                         start=True, stop=True)
                    pO3 = pO.rearrange("p (h c) -> p h c", c=DK + 1)
                    r4 = tp.tile([D, NH], F32, tag="r4", name=f"r4_{blk}_{b}")
                    nc.vector.reciprocal(r4[:], pO3[:, :, DK])
                    Osb = mp.tile([D, D], BF16, tag="Osb",
                                  name=f"Osb_{blk}_{b}")
                    for h in range(NH):
                        dst = Osb[:, bass.ts(h, DK)]
                        src = pO3[:, h, 0:DK]
                        if h < 2:
                            nc.vector.tensor_scalar(dst, src, r4[:, h:h + 1],
                                                    None, ALU.mult)
                        else:
                            nc.scalar.activation(dst, src, ACTF.Copy,
                                                 scale=r4[:, h:h + 1])
                    nc.tensor.transpose(ptO[:, bass.ts(b, L)], Osb[:], idb[:])
                if KLVL <= 3:
                    nc.sync.dma_start(
                        out_d[b0:b0 + NB, :, :].rearrange("b d l -> d b l"),
                        ycur.rearrange("p (n l) -> p n l", l=L))
                    continue
                OT = mp.tile([D, NB * L], BF16, tag="OT", name=f"OT_{blk}")
                nc.scalar.activation(OT[:], ptO[:], ACTF.Copy)

                pat = ppb.tile([D, NB * L], F32, tag="ph", name=f"pat_{blk}")
                for b in range(NB):
                    nc.tensor.matmul(pat[:, bass.ts(b, L)],
                                     OT[:, bass.ts(b, L)], wo[:],
                                     start=True, stop=True)
                if ext is not None and not bo_zero:
                    tb = trp.tile([D, NB * L], F32, tag="tb",
                                  name=f"tb_{blk}")
                    for b in range(NB):
                        nc.vector.tensor_tensor(tb[:, bass.ts(b, L)],
                                                pat[:, bass.ts(b, L)],
                                                ext[:, L:2 * L], ALU.add)
                    patsrc = tb
                else:
                    patsrc = pat
                ynew = yp.tile([D, NB * L], F32, tag="y", name=f"y5_{blk}")
                nc.vector.tensor_tensor(ynew[:], patsrc[:], ycur[:], ALU.add)
                y3 = ynew.rearrange("p (n l) -> p n l", l=L)
                st = stp.tile([D, 2 * NB], F32, tag="st", name=f"st5_{blk}")
                nc.vector.tensor_reduce(st[:, 0:NB], y3,
                                        mybir.AxisListType.X, ALU.add)
                stats_sq(st, y3, blk, 5)
                sc = finalize_norm(st)
                u5 = mp.tile([D, NB * L], BF16, tag="u5", name=f"u5_{blk}")
                u53 = u5.rearrange("p (n l) -> p n l", l=L)
                apply_norm(u53, slice(0, L), y3, sc, 5, blk)
                ycur = ynew

                # ---- fc + relu + residual -> out ----
                pf = ppb.tile([D, NB * L], F32, tag="ph", name=f"pf_{blk}")
                mm_banked(pf.rearrange("p (n l) -> p n l", l=L), fcw[:],
                          u5.rearrange("p (n l) -> p n l", l=L),
                          start=True, stop=True)
                tr = trp.tile([D, NB * L], BF16, tag="tr", name=f"trf_{blk}")
                nc.scalar.activation(tr[:], pf[:], ACTF.Relu,
                                     bias=bcol[:, 2:3])
                ostage = xp.tile([D, NB * L], F32, tag="os",
                                 name=f"os_{blk}")
                nc.vector.tensor_tensor(ostage[:], tr[:], ycur[:], ALU.add)
                nc.sync.dma_start(
                    out_d[b0:b0 + NB, :, :].rearrange("b d l -> d b l"),
                    ostage.rearrange("p (n l) -> p n l", l=L))

    nc.compile()
    return nc


def kernel(x, mask, dw_w, dw_b, pw_w, pw_b, normb_w, normb_b, norms_w, norms_b,
           norme_w, norme_b, Wq, bq, Wk, bk, Wv, bv, Wo, bo, fc_w, fc_b):
    global LAST_RESULT
    x = np.asarray(x, np.float32)
    mask = np.asarray(mask, np.int32)
    f = lambda a: np.asarray(a, np.float32)
    dw_w, dw_b, pw_w, pw_b = f(dw_w), f(dw_b), f(pw_w), f(pw_b)
    Wq, bq, Wk, bk, Wv, bv, Wo, bo = f(Wq), f(bq), f(Wk), f(bk), f(Wv), f(bv), f(Wo), f(bo)
    fc_w, fc_b = f(fc_w), f(fc_b)
    normb_w, normb_b = f(normb_w), f(normb_b)
    norms_w, norms_b = f(norms_w), f(norms_b)
    norme_w, norme_b = f(norme_w), f(norme_b)

    # ---- host-side constant prep ----
    pe = _pos_encoding()
    cw = np.zeros((D, LC * KW * D), BF)
    for i in range(LC):
        for k in range(KW):
            mk = pw_w[i] * dw_w[i][:, k][None, :]        # [o, c]
            cw[:, (i * KW + k) * D:(i * KW + k + 1) * D] = mk.T.astype(BF)
    bcols = np.zeros((D, 8), np.float32)
    bcols[:, 0] = bq
    bcols[:, 1] = bk
    bcols[:, 2] = fc_b
    for i in range(LC):
        bcols[:, 3 + i] = pw_w[i] @ dw_b[i] + pw_b[i]
    bcols[:, 7] = 1.0

    aw = [normb_w] + [norms_w[i] for i in range(LC)] + [norme_w]
    ab = [normb_b] + [norms_b[i] for i in range(LC)] + [norme_b]
    aff_identity = [bool(np.all(w == 1.0) and np.all(b == 0.0))
                    for w, b in zip(aw, ab)]
    bv_zero = bool(np.all(bv == 0.0))
    bo_zero = bool(np.all(bo == 0.0))

    nc = _build(aff_identity, bv_zero, bo_zero)

    wmap = {
        "cw": cw,
        "bc": bcols,
        "pe": pe,
        "wq": Wq.T.astype(BF),
        "wk": Wk.T.astype(BF),
        "wv": Wv.T.astype(BF),
        "wo": Wo.T.astype(BF),
        "fc": fc_w.T.astype(BF),
        "idb": np.eye(D, dtype=BF),
        "idf": np.eye(NB, dtype=np.float32),
        "on1": np.ones((1, D), np.float32),
    }
    if not all(aff_identity):
        affarr = np.zeros((D, 12 * L), np.float32)
        for i in range(6):
            affarr[:, 2 * i * L:(2 * i + 1) * L] = aw[i]
            affarr[:, (2 * i + 1) * L:(2 * i + 2) * L] = ab[i]
        wmap["aff"] = affarr
    if not (bv_zero and bo_zero):
        extarr = np.zeros((D, 2 * L), np.float32)
        extarr[:, 0:L] = np.broadcast_to(bv[None, :], (D, L))
        extarr[:, L:2 * L] = np.broadcast_to(bo[None, :], (D, L))
        wmap["ext"] = extarr

    core_ids = list(range(NCORES))
    in_maps = []
    for c in core_ids:
        m = dict(wmap)
        m["xs"] = np.ascontiguousarray(x[c * BS:(c + 1) * BS])
        m["ms"] = np.ascontiguousarray(mask[c * BS:(c + 1) * BS]).astype(np.float32)
        in_maps.append(m)

    if os.environ.get("KSIM", "0") == "1":
        from concourse.bass_interp import CoreSim
        sim = CoreSim(nc, require_finite=False, require_nnan=False)
        for k, v in in_maps[0].items():
            sim.tensor(k)[:] = v
        sim.simulate(check_with_hw=False)
        out0 = np.array(sim.tensor("out"))
        out = np.concatenate([out0] + [np.zeros_like(out0)] * (NCORES - 1), axis=0)
        return out.astype(np.float32)
    trace = os.environ.get("KERNEL_TRACE", "0") == "1"
    try:
        try:
            res = run_bass_kernel_spmd(nc, in_maps, core_ids, trace=trace)
        except Exception:
            if not trace:
                raise
            sys.stderr.write("traced run failed; retrying without trace\n")
            res = run_bass_kernel_spmd(nc, in_maps, core_ids, trace=False)
        LAST_RESULT = res
        out = np.concatenate([res.results[c]["out"] for c in range(NCORES)], axis=0)
        return out.astype(np.float32)
    except Exception as e:
        sys.stderr.write(f"HW path failed ({e}); falling back to CoreSim\n")
        from concourse.bass_interp import CoreSim
        outs = []
        for c in range(NCORES):
            sim = CoreSim(nc, require_finite=False, require_nnan=False)
            for k, v in in_maps[c].items():
                sim.tensor(k)[:] = v
            sim.simulate(check_with_hw=False)
            outs.append(np.array(sim.tensor("out")))
        return np.concatenate(outs, axis=0).astype(np.float32)
